# revision 53
# baseline (speedup 1.0000x reference)
"""Trainium2 Bass kernel for nn_HT_56298431316042 (histogram_binning).

Computes  out = relu(image.reshape(32, 16384)) @ vote.reshape(16384, 16384) / 128
         -> reshape (2, 16, 128, 128)

Sharding: column-wise over the 16384 Hough bins -> 2048 bins per core, 8 cores,
no communication.

Strategy (v5, bit-packed DVE expansion + small fp8 stream):
  Streaming the binary vote matrix as fp8 costs 1 B/vote (33.5 MB/core,
  ~93 us of DMA at the 360 GB/s pipe).  Instead, 1664 of the 2048 per-core
  bins are bit-packed host-side (8 votes/byte -> 4.2 MB/core) and expanded
  on-chip by the vector engine; the remaining 384 bins stream as plain fp8
  to fill leftover DMA bandwidth.  For each packed bit-plane ONE fused
  uint16 tensor_scalar emits valid fp8 *bit patterns* directly:
     bits 0-3:  (v & mask) << 3    -> bytes 0x08/0x10/0x20/0x40
     bits 4-6:  (v & mask)         -> bytes 0x10/0x20/0x40 (already fp8)
  (16-bit ops hit the DVE 4x perf mode; byte lanes never carry across.)
  Expanded tiles are bitcast to fp8 and fed to DoubleRow matmuls against
  fp8-quantized x; planes are laid out so each PSUM bank holds one plane
  pair consumed by single N=512 matmuls.  PSUM "start" zeroes a whole
  512-column bank, so start/stop flags are managed per bank, not per plane.
  PSUM is copied out raw and descaled per-plane host-side (1/BIT_VALUE).

  Pipeline: SP streams packed pieces + x + V slices (ordered so the vector
  engine starts at ~4 us and never stalls); PE consumes expansion units and
  V slices merged by estimated availability; ACT copies each finished bank
  out of PSUM while later banks still accumulate; output ships in two DMA
  pieces so only the last bank sits on the critical tail.
"""

import numpy as np

import concourse.bass as bass
import concourse.bacc as bacc
import concourse.mybir as mybir
import concourse.tile as tile
from concourse.bass_utils import run_bass_kernel_spmd

NCORES = 8
B, C, ROWS, COLS, H, W = 2, 16, 128, 128, 128, 128
BC = B * C                      # 32 output rows
K = ROWS * COLS                 # 16384 contraction
NTOT = H * W                    # 16384 output bins
NPC = NTOT // NCORES            # 2048 bins per core
KC = K // 128                   # 128 k-chunks
CCP = KC // 2                   # 64 k-chunk pairs (DoubleRow)

# ---- tunables -------------------------------------------------------------
NB = NPC // 8                   # 256 bins per bit-plane
NB2 = NB // 2                   # uint16 elements per (cc, j) row of packed P
X_SCALE = 16.0                  # x quantization scale (hi/lo fp8 split)
NQ = 4                          # P load quarters == unit granularity
QCC = CCP // NQ                 # ccpairs per quarter unit
EX_BUFS = 5
OUT_SPLIT = 3 * 512             # first out-DMA piece covers banks 0-2
SV = 384                        # streamed fp8 columns (psum 1664:2048)
EP6 = 128                       # expanded columns of plane 6 (psum 1536:1664)
BIT_VALUE = [2.0 ** -6, 2.0 ** -5, 2.0 ** -3, 2.0,   # bits 0-3 (shl 3)
             2.0 ** -5, 2.0 ** -3, 2.0,              # bits 4-6 (and only)
             2.0]                                    # bit 7  (shr 1)
# ---------------------------------------------------------------------------

_nc_cache: dict[str, object] = {}
_LABELS: dict[str, list] = {}

f8 = mybir.dt.float8e4
u16 = mybir.dt.uint16
f32 = mybir.dt.float32


def _lab(eng, label):
    _LABELS.setdefault(eng, []).append(label)


def _build(mode=None) -> object:
    if "nc" in _nc_cache:
        return _nc_cache["nc"]

    nc = bacc.Bacc("TRN2", target_bir_lowering=False, debug=False,
                   num_devices=NCORES)
    x_dram = nc.dram_tensor("x", (128, CCP * 2 * 32), f8, kind="ExternalInput")
    p_dram = nc.dram_tensor("p", (8, 128, (CCP // 8) * 2 * NB2), u16,
                            kind="ExternalInput")
    v_dram = nc.dram_tensor("v", (NQ, 128, QCC * 2 * SV), f8,
                            kind="ExternalInput")
    o_dram = nc.dram_tensor("out", (32, NPC), f32, kind="ExternalOutput")

    A = mybir.AluOpType

    with tile.TileContext(nc) as tc:
        with tc.tile_pool(name="xp", bufs=1) as xp, \
             tc.tile_pool(name="ptp", bufs=1) as ptp, \
             tc.tile_pool(name="exd", bufs=EX_BUFS) as exd_pool, \
             tc.tile_pool(name="op", bufs=1) as op, \
             tc.tile_pool(name="pp", bufs=1, space="PSUM") as pp, \
             tc.tile_pool(name="pt", bufs=1, space="PSUM") as pt_psum:

            xt = xp.tile([128, CCP, 2, 32], f8, name="xt")
            pt = ptp.tile([128, CCP, 2, NB2], u16, name="pt")
            vt = ptp.tile([128, CCP, 2, SV], f8, name="vt")
            psum = pp.tile([32, NPC], f32, name="psum")
            ob = op.tile([32, NPC], f32, name="ob")
            tokbank = pt_psum.tile([1, 16], f32, name="tokbank")

            # ---- SP: packed P pieces + x (no deps) ----
            PC8 = CCP // 8
            for pc in range(8):
                if pc == 1:
                    _lab("sp", "dma_x")
                    nc.sync.dma_start(out=xt[:], in_=x_dram.ap())
                _lab("sp", f"dma_P{pc}")
                nc.sync.dma_start(out=pt[:, pc * PC8:(pc + 1) * PC8, :, :],
                                  in_=p_dram.ap()[pc])
                if pc >= 5:
                    q = pc - 5
                    _lab("sp", f"dma_V{q}")
                    nc.sync.dma_start(
                        out=vt[:, q * QCC:(q + 1) * QCC, :, :],
                        in_=v_dram.ap()[q])
            _lab("sp", "dma_V3")
            nc.sync.dma_start(out=vt[:, 3 * QCC:4 * QCC, :, :],
                              in_=v_dram.ap()[3])

            # ---- DVE: fused u16 ops; the two bit-planes of each PSUM bank
            # write the two halves of one paired EX tile, so the PE consumes
            # them as single N=512 matmuls (half the PE instructions).
            # q0 is split into eighths for the earliest possible start.
            ex_of = {}

            def expand_pair(pair, c0, c1):
                ex_t = exd_pool.tile([128, c1 - c0, 2, 2 * NB2], u16,
                                     name="ex", tag="exd")
                for sub in range(2):
                    bit = 2 * pair + sub
                    mask = (1 << bit) * 257
                    src_ap = pt[:, c0:c1, :, :]
                    dst = ex_t[:, :, :, sub * NB2:(sub + 1) * NB2]
                    _lab("dve", f"ex_{bit}_{c0}")
                    if bit <= 3:
                        nc.vector.tensor_scalar(dst, src_ap, mask, 3,
                                                A.bitwise_and,
                                                A.logical_shift_left)
                    elif bit <= 6:
                        nc.vector.tensor_scalar(dst, src_ap, mask, None,
                                                A.bitwise_and)
                    else:
                        nc.vector.tensor_scalar(dst, src_ap, mask, 1,
                                                A.bitwise_and,
                                                A.logical_shift_right)
                ex_of[(pair, c0)] = ex_t

            def expand_p6(c0, c1):
                # plane 6 is 128 columns: bit 6 of packed byte-cols [0:EP6)
                ex_t = exd_pool.tile([128, c1 - c0, 2, EP6 // 2], u16,
                                     name="ex6", tag="ex6")
                _lab("dve", f"ex_6_{c0}")
                nc.vector.tensor_scalar(ex_t[:],
                                        pt[:, c0:c1, :, 0:EP6 // 2],
                                        (1 << 6) * 257, None, A.bitwise_and)
                ex_of[(3, c0)] = ex_t

            E8 = CCP // 8
            units = []
            for e in range(2):                       # q0 as eighths
                units.append((3, e * E8, (e + 1) * E8))
                for pair in range(3):
                    units.append((pair, e * E8, (e + 1) * E8))
            for h in range(1, 4):                    # remaining quarters
                units.append((3, h * QCC, (h + 1) * QCC))
                for pair in range(3):
                    units.append((pair, h * QCC, (h + 1) * QCC))
            for pair, c0, c1 in units:
                if pair < 3:
                    expand_pair(pair, c0, c1)
                else:
                    expand_p6(c0, c1)

            # ---- PE: x gate, then matmuls in unit order ----
            _lab("pe", "xgate")
            nc.tensor.matmul(tokbank[:], lhsT=xt[:, 0, 0, 0:1],
                             rhs=xt[:, 0, 0, 0:16], start=True, stop=True)

            # PSUM 'start' zeroes the WHOLE 512-column bank: banks 0-2 are
            # plane pairs; bank 3 = expanded plane-6a + streamed columns,
            # one accumulation group each.
            # DMA completion estimates from the actual SP emission order
            DMA_NS_PER_B = 1.0 / 360.0
            t_cur = 2000.0
            t_piece, t_vq = {}, {}
            p_bytes = 128 * PC8 * 2 * NB2 * 2
            v_bytes = 128 * QCC * 2 * SV
            for pc in range(8):
                if pc == 1:
                    t_cur += 128 * CCP * 2 * 32 * DMA_NS_PER_B
                t_cur += p_bytes * DMA_NS_PER_B
                t_piece[pc] = t_cur
                if pc >= 5:
                    t_cur += v_bytes * DMA_NS_PER_B
                    t_vq[pc - 5] = t_cur
            t_cur += v_bytes * DMA_NS_PER_B
            t_vq[3] = t_cur

            ev = []
            cur_dve = 4300.0
            for pair, c0, c1 in units:
                need = t_piece[(c1 - 1) // PC8] + 900.0
                dcost = (c1 - c0) * 2 * NB2 * 1.042 * 0.25 * (
                    2 if pair < 3 else 0.5) + 120
                cur_dve = max(cur_dve, need) + dcost
                ev.append((cur_dve + 1000.0, "unit", (pair, c0, c1)))
            for q in range(NQ):
                ev.append((t_vq[q] + 2500.0, "vq", q))
            ev.sort(key=lambda e: e[0])

            bank_left = {pair: CCP for pair in range(4)}
            vq_left = NQ
            bank_seen = {}
            BK3_MMS = 2 * CCP

            def bump(bank, n=1):
                bank_seen[bank] = bank_seen.get(bank, 0) + n
                return bank_seen[bank]

            def bank_done(pair):
                base = pair * 512
                _lab("act", f"rcopy_{pair}")
                nc.scalar.copy(ob[:, base:base + 512],
                               psum[:, base:base + 512])
                if base + 512 == OUT_SPLIT:
                    # ship banks 0-2 while bank 3 finishes
                    _lab("sp", "outdma0")
                    nc.sync.dma_start(out=o_dram.ap()[:, 0:OUT_SPLIT],
                                      in_=ob[:, 0:OUT_SPLIT])

            for _, kind, idx in ev:
                if kind == "unit":
                    pair, c0, c1 = idx
                    if pair < 3:
                        exf8 = ex_of[(pair, c0)][:].bitcast(f8)
                        base = pair * 512
                        for ccl in range(c1 - c0):
                            n = bump(pair)
                            _lab("pe", f"mm_{pair}_{c0}_{ccl}")
                            nc.tensor.matmul(
                                psum[:, base:base + 512],
                                lhsT=xt[:, c0 + ccl, :, :],
                                rhs=exf8[:, ccl, :, :],
                                start=(n == 1), stop=(n == CCP),
                                perf_mode=mybir.MatmulPerfMode.DoubleRow)
                        bank_left[pair] -= (c1 - c0)
                        if bank_left[pair] == 0:
                            bank_done(pair)
                    else:
                        exf8 = ex_of[(3, c0)][:].bitcast(f8)
                        for ccl in range(c1 - c0):
                            n = bump(3)
                            _lab("pe", f"mm6_{c0}_{ccl}")
                            nc.tensor.matmul(
                                psum[:, 1536:1536 + EP6],
                                lhsT=xt[:, c0 + ccl, :, :],
                                rhs=exf8[:, ccl, :, :],
                                start=(n == 1), stop=(n == BK3_MMS),
                                perf_mode=mybir.MatmulPerfMode.DoubleRow)
                        bank_left[3] -= (c1 - c0)
                else:
                    q = idx
                    for ccl in range(QCC):
                        n = bump(3)
                        _lab("pe", f"mmv_{q}_{ccl}")
                        nc.tensor.matmul(
                            psum[:, 1536 + EP6:NPC],
                            lhsT=xt[:, q * QCC + ccl, :, :],
                            rhs=vt[:, q * QCC + ccl, :, :],
                            start=(n == 1), stop=(n == BK3_MMS),
                            perf_mode=mybir.MatmulPerfMode.DoubleRow)
                    vq_left -= 1
            _lab("act", "rcopy_3")
            nc.scalar.copy(ob[:, 1536:NPC], psum[:, 1536:NPC])

            # ---- epilogue: last out piece (SP: shorter DGE path) ----
            _lab("sp", "outdma1")
            nc.sync.dma_start(out=o_dram.ap()[:, OUT_SPLIT:NPC],
                              in_=ob[:, OUT_SPLIT:NPC])

    nc.finalize()
    _nc_cache["nc"] = nc
    return nc


def _prep_inputs(image: np.ndarray, vote_index: np.ndarray):
    np_f8 = mybir.dt.np(f8)

    x = np.maximum(image.reshape(BC, K).astype(np.float32), 0.0) * X_SCALE
    hi = x.astype(np_f8)
    xarr = np.ascontiguousarray(
        hi.reshape(BC, CCP, 2, 128).transpose(3, 1, 2, 0)
    ).reshape(128, CCP * 2 * 32)

    v2 = vote_index.reshape(K, NTOT)
    in_maps = []
    for c in range(NCORES):
        cols = v2[:, c * NPC:(c + 1) * NPC]
        be = cols[:, :6 * NB].astype(np.uint8).reshape(K, 6, NB)
        bytes_ = np.zeros((K, NB), dtype=np.uint8)
        for i in range(6):
            bytes_ |= be[:, i, :] << i
        bytes_[:, 0:EP6] |= \
            cols[:, 6 * NB:6 * NB + EP6].astype(np.uint8) << 6
        pb = np.ascontiguousarray(
            bytes_.reshape(CCP, 2, 128, NB).transpose(2, 0, 1, 3))
        pu16 = pb.reshape(128, CCP, 2, NB2, 2).view(np.uint16)[..., 0]
        parr = np.ascontiguousarray(
            pu16.reshape(128, 8, CCP // 8, 2, NB2).transpose(1, 0, 2, 3, 4)
        ).reshape(8, 128, (CCP // 8) * 2 * NB2)
        vs = cols[:, NPC - SV:].astype(np_f8)
        varr = np.ascontiguousarray(
            vs.reshape(CCP, 2, 128, SV).transpose(2, 0, 1, 3))
        varr = np.ascontiguousarray(
            varr.reshape(128, NQ, QCC, 2, SV).transpose(1, 0, 2, 3, 4)
        ).reshape(NQ, 128, QCC * 2 * SV)
        in_maps.append({"x": xarr, "p": parr, "v": varr})
    return in_maps


_DESCALE = None


def _descale_vec():
    global _DESCALE
    if _DESCALE is None:
        d = np.ones(NPC, dtype=np.float32)
        for i in range(6):
            d[i * NB:(i + 1) * NB] = 1.0 / BIT_VALUE[i]
        d[6 * NB:6 * NB + EP6] = 0.5      # plane 6a: AND-only, fp8 value 2.0
        d[6 * NB + EP6:] = 1.0            # streamed raw 0/1
        _DESCALE = d / (COLS * X_SCALE)
    return _DESCALE


def _run(image, vote_index, mode=None, **run_kwargs):
    nc = _build()
    in_maps = _prep_inputs(np.asarray(image), np.asarray(vote_index))
    res = run_bass_kernel_spmd(nc, in_maps, core_ids=list(range(NCORES)),
                               **run_kwargs)
    ds = _descale_vec()
    outs = []
    for r in res.results:
        outs.append(r["out"] * ds)
    out = np.concatenate(outs, axis=1)
    return out.reshape(B, C, H, W).astype(np.float32), res


def kernel(image: np.ndarray, vote_index: np.ndarray) -> np.ndarray:
    out, _ = _run(image, vote_index)
    return out


MODE = "v5"


# revision 55
# speedup vs baseline: 1.0021x; 1.0021x over previous
"""Trainium2 Bass kernel for nn_HT_56298431316042 (histogram_binning).

Computes  out = relu(image.reshape(32, 16384)) @ vote.reshape(16384, 16384) / 128
         -> reshape (2, 16, 128, 128)

Sharding: column-wise over the 16384 Hough bins -> 2048 bins per core, 8 cores,
no communication.

Strategy (v5, bit-packed DVE expansion + small fp8 stream):
  Streaming the binary vote matrix as fp8 costs 1 B/vote (33.5 MB/core,
  ~93 us of DMA at the 360 GB/s pipe).  Instead, 1664 of the 2048 per-core
  bins are bit-packed host-side (8 votes/byte -> 4.2 MB/core) and expanded
  on-chip by the vector engine; the remaining 384 bins stream as plain fp8
  to fill leftover DMA bandwidth.  For each packed bit-plane ONE fused
  uint16 tensor_scalar emits valid fp8 *bit patterns* directly:
     bits 0-3:  (v & mask) << 3    -> bytes 0x08/0x10/0x20/0x40
     bits 4-6:  (v & mask)         -> bytes 0x10/0x20/0x40 (already fp8)
  (16-bit ops hit the DVE 4x perf mode; byte lanes never carry across.)
  Expanded tiles are bitcast to fp8 and fed to DoubleRow matmuls against
  fp8-quantized x; planes are laid out so each PSUM bank holds one plane
  pair consumed by single N=512 matmuls.  PSUM "start" zeroes a whole
  512-column bank, so start/stop flags are managed per bank, not per plane.
  PSUM is copied out raw and descaled per-plane host-side (1/BIT_VALUE).

  Pipeline: SP streams packed pieces + x + V slices (ordered so the vector
  engine starts at ~4 us and never stalls); PE consumes expansion units and
  V slices merged by estimated availability; ACT copies each finished bank
  out of PSUM while later banks still accumulate; output ships in two DMA
  pieces so only the last bank sits on the critical tail.
"""

import numpy as np

import concourse.bass as bass
import concourse.bacc as bacc
import concourse.mybir as mybir
import concourse.tile as tile
from concourse.bass_utils import run_bass_kernel_spmd

NCORES = 8
B, C, ROWS, COLS, H, W = 2, 16, 128, 128, 128, 128
BC = B * C                      # 32 output rows
K = ROWS * COLS                 # 16384 contraction
NTOT = H * W                    # 16384 output bins
NPC = NTOT // NCORES            # 2048 bins per core
KC = K // 128                   # 128 k-chunks
CCP = KC // 2                   # 64 k-chunk pairs (DoubleRow)

# ---- tunables -------------------------------------------------------------
NB = NPC // 8                   # 256 bins per bit-plane
NB2 = NB // 2                   # uint16 elements per (cc, j) row of packed P
X_SCALE = 16.0                  # x quantization scale (hi/lo fp8 split)
NQ = 4                          # P load quarters == unit granularity
QCC = CCP // NQ                 # ccpairs per quarter unit
EX_BUFS = 5
OUT_SPLIT = 3 * 512             # first out-DMA piece covers banks 0-2
SV = 384                        # streamed fp8 columns (psum 1664:2048)
EP6 = 128                       # expanded columns of plane 6 (psum 1536:1664)
BIT_VALUE = [2.0 ** -6, 2.0 ** -5, 2.0 ** -3, 2.0,   # bits 0-3 (shl 3)
             2.0 ** -5, 2.0 ** -3, 2.0,              # bits 4-6 (and only)
             2.0]                                    # bit 7  (shr 1)
# ---------------------------------------------------------------------------

_nc_cache: dict[str, object] = {}
_LABELS: dict[str, list] = {}

f8 = mybir.dt.float8e4
u16 = mybir.dt.uint16
f32 = mybir.dt.float32
bf16 = mybir.dt.bfloat16


def _lab(eng, label):
    _LABELS.setdefault(eng, []).append(label)


def _build(mode=None) -> object:
    if "nc" in _nc_cache:
        return _nc_cache["nc"]

    nc = bacc.Bacc("TRN2", target_bir_lowering=False, debug=False,
                   num_devices=NCORES)
    x_dram = nc.dram_tensor("x", (128, CCP * 2 * 32), f8, kind="ExternalInput")
    p_dram = nc.dram_tensor("p", (8, 128, (CCP // 8) * 2 * NB2), u16,
                            kind="ExternalInput")
    v_dram = nc.dram_tensor("v", (NQ, 128, QCC * 2 * SV), f8,
                            kind="ExternalInput")
    o_dram = nc.dram_tensor("out", (32, NPC), bf16, kind="ExternalOutput")

    A = mybir.AluOpType

    with tile.TileContext(nc) as tc:
        with tc.tile_pool(name="xp", bufs=1) as xp, \
             tc.tile_pool(name="ptp", bufs=1) as ptp, \
             tc.tile_pool(name="exd", bufs=EX_BUFS) as exd_pool, \
             tc.tile_pool(name="op", bufs=1) as op, \
             tc.tile_pool(name="pp", bufs=1, space="PSUM") as pp, \
             tc.tile_pool(name="pt", bufs=1, space="PSUM") as pt_psum:

            xt = xp.tile([128, CCP, 2, 32], f8, name="xt")
            pt = ptp.tile([128, CCP, 2, NB2], u16, name="pt")
            vt = ptp.tile([128, CCP, 2, SV], f8, name="vt")
            psum = pp.tile([32, NPC], f32, name="psum")
            ob = op.tile([32, NPC], bf16, name="ob")
            tokbank = pt_psum.tile([1, 16], f32, name="tokbank")

            # ---- SP: packed P pieces + x (no deps) ----
            PC8 = CCP // 8
            for pc in range(8):
                if pc == 1:
                    _lab("sp", "dma_x")
                    nc.sync.dma_start(out=xt[:], in_=x_dram.ap())
                _lab("sp", f"dma_P{pc}")
                nc.sync.dma_start(out=pt[:, pc * PC8:(pc + 1) * PC8, :, :],
                                  in_=p_dram.ap()[pc])
                if pc >= 5:
                    q = pc - 5
                    _lab("sp", f"dma_V{q}")
                    nc.sync.dma_start(
                        out=vt[:, q * QCC:(q + 1) * QCC, :, :],
                        in_=v_dram.ap()[q])
            _lab("sp", "dma_V3")
            nc.sync.dma_start(out=vt[:, 3 * QCC:4 * QCC, :, :],
                              in_=v_dram.ap()[3])

            # ---- DVE: fused u16 ops; the two bit-planes of each PSUM bank
            # write the two halves of one paired EX tile, so the PE consumes
            # them as single N=512 matmuls (half the PE instructions).
            # q0 is split into eighths for the earliest possible start.
            ex_of = {}

            def expand_pair(pair, c0, c1):
                ex_t = exd_pool.tile([128, c1 - c0, 2, 2 * NB2], u16,
                                     name="ex", tag="exd")
                for sub in range(2):
                    bit = 2 * pair + sub
                    mask = (1 << bit) * 257
                    src_ap = pt[:, c0:c1, :, :]
                    dst = ex_t[:, :, :, sub * NB2:(sub + 1) * NB2]
                    _lab("dve", f"ex_{bit}_{c0}")
                    if bit <= 3:
                        nc.vector.tensor_scalar(dst, src_ap, mask, 3,
                                                A.bitwise_and,
                                                A.logical_shift_left)
                    elif bit <= 6:
                        nc.vector.tensor_scalar(dst, src_ap, mask, None,
                                                A.bitwise_and)
                    else:
                        nc.vector.tensor_scalar(dst, src_ap, mask, 1,
                                                A.bitwise_and,
                                                A.logical_shift_right)
                ex_of[(pair, c0)] = ex_t

            def expand_p6(c0, c1):
                # plane 6 is 128 columns: bit 6 of packed byte-cols [0:EP6)
                ex_t = exd_pool.tile([128, c1 - c0, 2, EP6 // 2], u16,
                                     name="ex6", tag="ex6")
                _lab("dve", f"ex_6_{c0}")
                nc.vector.tensor_scalar(ex_t[:],
                                        pt[:, c0:c1, :, 0:EP6 // 2],
                                        (1 << 6) * 257, None, A.bitwise_and)
                ex_of[(3, c0)] = ex_t

            E8 = CCP // 8
            units = []
            for e in range(2):                       # q0 as eighths
                units.append((3, e * E8, (e + 1) * E8))
                for pair in range(3):
                    units.append((pair, e * E8, (e + 1) * E8))
            for h in range(1, 4):                    # remaining quarters
                units.append((3, h * QCC, (h + 1) * QCC))
                for pair in range(3):
                    units.append((pair, h * QCC, (h + 1) * QCC))
            for pair, c0, c1 in units:
                if pair < 3:
                    expand_pair(pair, c0, c1)
                else:
                    expand_p6(c0, c1)

            # ---- PE: x gate, then matmuls in unit order ----
            _lab("pe", "xgate")
            nc.tensor.matmul(tokbank[:], lhsT=xt[:, 0, 0, 0:1],
                             rhs=xt[:, 0, 0, 0:16], start=True, stop=True)

            # PSUM 'start' zeroes the WHOLE 512-column bank: banks 0-2 are
            # plane pairs; bank 3 = expanded plane-6a + streamed columns,
            # one accumulation group each.
            # DMA completion estimates from the actual SP emission order
            DMA_NS_PER_B = 1.0 / 360.0
            t_cur = 2000.0
            t_piece, t_vq = {}, {}
            p_bytes = 128 * PC8 * 2 * NB2 * 2
            v_bytes = 128 * QCC * 2 * SV
            for pc in range(8):
                if pc == 1:
                    t_cur += 128 * CCP * 2 * 32 * DMA_NS_PER_B
                t_cur += p_bytes * DMA_NS_PER_B
                t_piece[pc] = t_cur
                if pc >= 5:
                    t_cur += v_bytes * DMA_NS_PER_B
                    t_vq[pc - 5] = t_cur
            t_cur += v_bytes * DMA_NS_PER_B
            t_vq[3] = t_cur

            ev = []
            cur_dve = 4300.0
            for pair, c0, c1 in units:
                need = t_piece[(c1 - 1) // PC8] + 900.0
                dcost = (c1 - c0) * 2 * NB2 * 1.042 * 0.25 * (
                    2 if pair < 3 else 0.5) + 120
                cur_dve = max(cur_dve, need) + dcost
                ev.append((cur_dve + 1000.0, "unit", (pair, c0, c1)))
            for q in range(NQ):
                ev.append((t_vq[q] + 2500.0, "vq", q))
            ev.sort(key=lambda e: e[0])

            bank_left = {pair: CCP for pair in range(4)}
            vq_left = NQ
            bank_seen = {}
            BK3_MMS = 2 * CCP

            def bump(bank, n=1):
                bank_seen[bank] = bank_seen.get(bank, 0) + n
                return bank_seen[bank]

            def bank_done(pair):
                base = pair * 512
                _lab("act", f"rcopy_{pair}")
                nc.scalar.copy(ob[:, base:base + 512],
                               psum[:, base:base + 512])
                if base + 512 == OUT_SPLIT:
                    # ship banks 0-2 while bank 3 finishes
                    _lab("sp", "outdma0")
                    nc.sync.dma_start(out=o_dram.ap()[:, 0:OUT_SPLIT],
                                      in_=ob[:, 0:OUT_SPLIT])

            for _, kind, idx in ev:
                if kind == "unit":
                    pair, c0, c1 = idx
                    if pair < 3:
                        exf8 = ex_of[(pair, c0)][:].bitcast(f8)
                        base = pair * 512
                        for ccl in range(c1 - c0):
                            n = bump(pair)
                            _lab("pe", f"mm_{pair}_{c0}_{ccl}")
                            nc.tensor.matmul(
                                psum[:, base:base + 512],
                                lhsT=xt[:, c0 + ccl, :, :],
                                rhs=exf8[:, ccl, :, :],
                                start=(n == 1), stop=(n == CCP),
                                perf_mode=mybir.MatmulPerfMode.DoubleRow)
                        bank_left[pair] -= (c1 - c0)
                        if bank_left[pair] == 0:
                            bank_done(pair)
                    else:
                        exf8 = ex_of[(3, c0)][:].bitcast(f8)
                        for ccl in range(c1 - c0):
                            n = bump(3)
                            _lab("pe", f"mm6_{c0}_{ccl}")
                            nc.tensor.matmul(
                                psum[:, 1536:1536 + EP6],
                                lhsT=xt[:, c0 + ccl, :, :],
                                rhs=exf8[:, ccl, :, :],
                                start=(n == 1), stop=(n == BK3_MMS),
                                perf_mode=mybir.MatmulPerfMode.DoubleRow)
                        bank_left[3] -= (c1 - c0)
                else:
                    q = idx
                    for ccl in range(QCC):
                        n = bump(3)
                        _lab("pe", f"mmv_{q}_{ccl}")
                        nc.tensor.matmul(
                            psum[:, 1536 + EP6:NPC],
                            lhsT=xt[:, q * QCC + ccl, :, :],
                            rhs=vt[:, q * QCC + ccl, :, :],
                            start=(n == 1), stop=(n == BK3_MMS),
                            perf_mode=mybir.MatmulPerfMode.DoubleRow)
                    vq_left -= 1
            _lab("act", "rcopy_3")
            nc.scalar.copy(ob[:, 1536:NPC], psum[:, 1536:NPC])

            # ---- epilogue: last out piece (SP: shorter DGE path) ----
            _lab("sp", "outdma1")
            nc.sync.dma_start(out=o_dram.ap()[:, OUT_SPLIT:NPC],
                              in_=ob[:, OUT_SPLIT:NPC])

    nc.finalize()
    _nc_cache["nc"] = nc
    return nc


def _prep_inputs(image: np.ndarray, vote_index: np.ndarray):
    np_f8 = mybir.dt.np(f8)

    x = np.maximum(image.reshape(BC, K).astype(np.float32), 0.0) * X_SCALE
    hi = x.astype(np_f8)
    xarr = np.ascontiguousarray(
        hi.reshape(BC, CCP, 2, 128).transpose(3, 1, 2, 0)
    ).reshape(128, CCP * 2 * 32)

    v2 = vote_index.reshape(K, NTOT)
    in_maps = []
    for c in range(NCORES):
        cols = v2[:, c * NPC:(c + 1) * NPC]
        be = cols[:, :6 * NB].astype(np.uint8).reshape(K, 6, NB)
        bytes_ = np.zeros((K, NB), dtype=np.uint8)
        for i in range(6):
            bytes_ |= be[:, i, :] << i
        bytes_[:, 0:EP6] |= \
            cols[:, 6 * NB:6 * NB + EP6].astype(np.uint8) << 6
        pb = np.ascontiguousarray(
            bytes_.reshape(CCP, 2, 128, NB).transpose(2, 0, 1, 3))
        pu16 = pb.reshape(128, CCP, 2, NB2, 2).view(np.uint16)[..., 0]
        parr = np.ascontiguousarray(
            pu16.reshape(128, 8, CCP // 8, 2, NB2).transpose(1, 0, 2, 3, 4)
        ).reshape(8, 128, (CCP // 8) * 2 * NB2)
        vs = cols[:, NPC - SV:].astype(np_f8)
        varr = np.ascontiguousarray(
            vs.reshape(CCP, 2, 128, SV).transpose(2, 0, 1, 3))
        varr = np.ascontiguousarray(
            varr.reshape(128, NQ, QCC, 2, SV).transpose(1, 0, 2, 3, 4)
        ).reshape(NQ, 128, QCC * 2 * SV)
        in_maps.append({"x": xarr, "p": parr, "v": varr})
    return in_maps


_DESCALE = None


def _descale_vec():
    global _DESCALE
    if _DESCALE is None:
        d = np.ones(NPC, dtype=np.float32)
        for i in range(6):
            d[i * NB:(i + 1) * NB] = 1.0 / BIT_VALUE[i]
        d[6 * NB:6 * NB + EP6] = 0.5      # plane 6a: AND-only, fp8 value 2.0
        d[6 * NB + EP6:] = 1.0            # streamed raw 0/1
        _DESCALE = d / (COLS * X_SCALE)
    return _DESCALE


def _run(image, vote_index, mode=None, **run_kwargs):
    nc = _build()
    in_maps = _prep_inputs(np.asarray(image), np.asarray(vote_index))
    res = run_bass_kernel_spmd(nc, in_maps, core_ids=list(range(NCORES)),
                               **run_kwargs)
    ds = _descale_vec()
    outs = []
    for r in res.results:
        outs.append(r["out"].astype(np.float32) * ds)
    out = np.concatenate(outs, axis=1)
    return out.reshape(B, C, H, W).astype(np.float32), res


def kernel(image: np.ndarray, vote_index: np.ndarray) -> np.ndarray:
    out, _ = _run(image, vote_index)
    return out


MODE = "v5"
